# revision 13
# baseline (speedup 1.0000x reference)
"""Trainium2 Bass kernel for nn_ConstantCurrentLIFEncoder.

Reference semantics (norse constant_current_lif_encode, f32):
    v' = v + f32(0.1) * ((0 - v) + I);  z = (v' > 1);  v = v' - z*v'
for 100 steps from v=0, returning spikes [100, 32, 8192].

A spike resets v to exactly 0.0 (the initial state) and I is constant,
so each neuron's spike train is exactly periodic with period
    p = 1 + floor(ln((I-1)/I) / ln(1 - f32(0.1)))    (no spikes if I <= 1;
periods are clamped to 129 since anything > 100 never fires in-window).
Instead of a 100-step sequential scan the kernel computes p per neuron
in closed form (a handful of small ops on [128, 256]) and generates the
output from the periodicity:  z[t, n] = 1  iff  p_n | (t+1), evaluated as

    z[t, n] = floor((t+1.5) r_n) - floor((t+0.5) r_n),   r = 1/p

(the count of integers in ((t+0.5)r, (t+1.5)r], which is 0 or 1 since
r <= 1/7).  With x[t] = (t+0.5) r_n for t = 0..100 this is
z[t] = F[t+1] - F[t], F = floor(x).  Compute engines cannot read at a
+1 partition offset (the ISA cannot encode it), but DMA moves across
partitions freely, so the shifted copy G[t] = F[t+1] is produced by a
small SBUF->SBUF DMA on the ScalarE hardware queue (the output stream
runs on the Sync queue in parallel):

  * PE      : x = (t+0.5)-row (bf16) x r-row (bf16), t = 0..100; K=1
              outer products -> PSUM f32 [101, 512] per bank.  bf16
              runs 1 cycle/row (fp32 is 4) and the products are exact
              in f32; r's bf16 rounding shifts x by <= 0.196 r, margin
              to any decision boundary is 0.5 r.
  * floor   : F = RN_bf16(x + 143.5) = 144 + floor(x) (add-magic trick;
              exact: F in [144, 159], bf16 ulp there is 1).  Tiles are
              split between ScalarE (activation Copy + bias) and DVE
              (tensor_scalar add) to balance engine load.
  * shift   : G = F[1:101] via SBUF->SBUF DMA (partition shift).
  * diff    : z = G - F (tensor_tensor subtract, all-bf16 SBUF operands
              -> 2x DVE mode), some tiles on GpSimd.
  * DMA     : z -> DRAM as bf16 (half the bytes of f32); the host casts
              back to f32 (0.0/1.0 are exact in bf16).

PE K=1 operands must sit at partition bases {0,64}, so r is relayouted
from [partition, group] to two flat 16384-wide rows via a small DRAM
relayout (DMA transpose + SBUF->SBUF row gather), one-time cost.

Sharding: pure data parallel over the neuron axis; core c handles
columns [c*1024, (c+1)*1024) of the [32, 8192] input. No collectives.
"""

import math

import numpy as np

SEQ = 100
SEQ1 = SEQ + 1
B = 32
N = 8192
NCORES = 8
NPC = N // NCORES  # neurons (last-axis columns) per core
M = B * NPC  # flat neurons per core = 32768
P = 128
J = M // P  # 128-neuron groups per core = 256

# Effective per-step decay of (v - I): v' - I = (1 - f32(0.1)) * (v - I).
DECAY = 1.0 - float(np.float32(0.1))
INV_LN_D = float(np.float32(1.0 / math.log(DECAY)))
C23 = float(2.0**23)
# RN_bf16(x + 143.5) = 144 + floor(x) for x in (0, 16) away from integers:
# v = x + 143.5 and the result 144 + floor(x) <= 159 both stay inside the
# [128, 256) binade where bf16's ulp is exactly 1 (7 stored mantissa bits).
MAGIC = 143.5
KCLAMP = 128.0

TW = 2048  # neuron columns per compute tile (4 PSUM banks)
NT = M // TW  # 16 tiles per core
# tiles whose floor runs on DVE instead of ScalarE (load balance)
DVE_FLOOR = {2, 5, 8, 11, 14}
# tiles whose diff runs on GpSimd instead of DVE
GP_DIFF = {1, 5, 9, 13}

_CACHE: dict = {}


def _build_nc():
    import concourse.bacc as bacc
    import concourse.mybir as mybir
    from concourse import tile

    f32 = mybir.dt.float32
    bf16 = mybir.dt.bfloat16
    Alu = mybir.AluOpType
    Act = mybir.ActivationFunctionType

    nc = bacc.Bacc("TRN2", target_bir_lowering=False, debug=False)
    cur = nc.dram_tensor("cur", [P, J], f32, kind="ExternalInput")
    # (t+0.5) rows for t=0..100, replicated at partitions 0/64 (PE
    # quadrant bases; lhsT must share its base with rhs)
    trow = nc.dram_tensor("trow", [P, SEQ1], f32, kind="ExternalInput")
    out = nc.dram_tensor("out", [SEQ, M], bf16, kind="ExternalOutput")

    with tile.TileContext(nc) as tc:
        with (
            tc.tile_pool(name="consts", bufs=1) as consts,
            tc.tile_pool(name="pp", bufs=2, space="PSUM") as pp,
            tc.tile_pool(name="fp", bufs=3) as fp,
            tc.tile_pool(name="gp", bufs=2) as gp,
            tc.tile_pool(name="zp", bufs=2) as zp,
        ):
            cur_s = consts.tile([P, J], f32)
            trow_s = consts.tile([P, SEQ1], f32)
            trow16 = consts.tile([P, SEQ1], bf16)
            t1 = consts.tile([P, J], f32)
            t2 = consts.tile([P, J], f32)
            rbuf = consts.tile([P, J], f32)
            rbuf16 = consts.tile([P, J], bf16)
            rT = [
                consts.tile([P, P], bf16, name=f"rT{c}", tag=f"rT{c}")
                for c in range(J // P)
            ]
            rquad = consts.tile([65, M // 2], bf16)

            # Warm the ScalarE activation table (Ln) before the input DMA
            # lands so the ~2.5us table load overlaps the preamble DMAs.
            nc.gpsimd.memset(t1[:, 0:8], 1.0)
            nc.scalar.activation(t2[:, 0:8], t1[:, 0:8], Act.Ln)

            nc.sync.dma_start(cur_s[:], cur[:, :])
            nc.sync.dma_start(trow_s[:], trow[:, :])
            nc.scalar.activation(trow16[:], trow_s[:], Act.Copy)

            # ---- per-neuron period p, rate r = 1/p ----
            # Processed in two 128-column halves so the first half's r row
            # (neurons 0..16383) is ready ~5us earlier and the matmul stream
            # starts while the second half's chain still runs.  Each half's
            # columns of rbuf are exactly one DMA-transpose chunk.
            for h in range(2):
                cs = slice(h * P, (h + 1) * P)
                t1h, t2h = t1[:, cs], t2[:, cs]
                nc.vector.reciprocal(t1h, cur_s[:, cs])  # 1/I
                nc.vector.tensor_scalar(t2h, cur_s[:, cs], -1.0, None, Alu.add)
                nc.vector.tensor_tensor(t2h, t2h, t1h, Alu.mult)  # (I-1)/I
                nc.vector.tensor_scalar(t2h, t2h, 1e-38, None, Alu.max)
                nc.scalar.activation(t1h, t2h, Act.Ln)
                # k = ln(a)/ln(d), clamped so p <= 129 (never fires in-window)
                nc.vector.tensor_scalar(
                    t1h, t1h, INV_LN_D, KCLAMP, Alu.mult, Alu.min
                )
                # floor(k) via the round-to-nearest add-2^23 trick on k - 0.5
                nc.vector.tensor_scalar(
                    t1h, t1h, C23 - 0.5, C23, Alu.add, Alu.subtract
                )
                nc.vector.tensor_scalar(t2h, t1h, 1.0, None, Alu.add)  # p
                nc.vector.reciprocal(rbuf[:, cs], t2h)  # r = 1/p, f32
                # PE needs flat bf16 rows at quadrant bases: cast to bf16,
                # DMA-transpose, bounce through DRAM, reload the
                # 16384-neuron half into partition 64*h.
                nc.scalar.activation(rbuf16[:, cs], rbuf[:, cs], Act.Copy)
                nc.sync.dma_start_transpose(rT[h][:], rbuf16[:, cs])
                nc.sync.dma_start(rquad[64 * h : 64 * h + 1, :], rT[h][:, :])

            # ---- spike-train generation, output-layout [time, neuron] ----
            # Floors are issued as two half-tiles so the first half starts
            # while the PE is still filling the second (frees the PSUM slot
            # earlier, keeping the PE fed).
            HW2 = TW // 2
            z = None
            for w in range(NT):
                half, off = divmod(w * TW, M // 2)
                b = 64 * half
                x = pp.tile([SEQ1, TW], f32, name="x", tag="x")
                for s in range(TW // 512):
                    nc.tensor.matmul(
                        x[:, s * 512 : (s + 1) * 512],
                        trow16[b : b + 1, 0:SEQ1],
                        rquad[b : b + 1, off + s * 512 : off + (s + 1) * 512],
                        start=True,
                        stop=True,
                    )
                fb = fp.tile([SEQ1, TW], bf16)
                for h in range(2):
                    fs = fb[:, h * HW2 : (h + 1) * HW2]
                    xsl = x[:, h * HW2 : (h + 1) * HW2]
                    if w in DVE_FLOOR:
                        nc.vector.tensor_scalar(fs, xsl, MAGIC, None, Alu.add)
                    else:
                        nc.scalar.activation(fs, xsl, Act.Copy, bias=MAGIC)
                g = gp.tile([SEQ, TW], bf16)
                nc.sync.dma_start(g[:], fb[1:SEQ1, :])
                if w % 2 == 0:
                    z = zp.tile([SEQ, 2 * TW], bf16)
                zs = z[:, (w % 2) * TW : (w % 2 + 1) * TW]
                eng = nc.gpsimd if w in GP_DIFF else nc.vector
                eng.tensor_tensor(zs, g[:], fb[0:SEQ, :], Alu.subtract)
                if w % 2 == 1:
                    nc.sync.dma_start(
                        out[0:SEQ, (w - 1) * TW : (w + 1) * TW], z[:]
                    )

    nc.compile()
    return nc


def _get_nc():
    if "nc" not in _CACHE:
        _CACHE["nc"] = _build_nc()
    return _CACHE["nc"]


def _in_maps(input_currents: np.ndarray):
    trow = np.zeros((P, SEQ1), dtype=np.float32)
    for b in (0, 64):
        trow[b, :] = np.arange(SEQ1, dtype=np.float32) + np.float32(0.5)
    maps = []
    for c in range(NCORES):
        shard = np.asarray(input_currents[:, c * NPC : (c + 1) * NPC], dtype=np.float32)
        # device layout: cur[p, j] = flat neuron j*128 + p
        cur = np.ascontiguousarray(shard.reshape(M).reshape(J, P).T)
        maps.append({"cur": cur, "trow": trow})
    return maps


def run_spmd(input_currents: np.ndarray, **kwargs):
    from concourse.bass_utils import run_bass_kernel_spmd

    nc = _get_nc()
    res = run_bass_kernel_spmd(
        nc, _in_maps(input_currents), core_ids=list(range(NCORES)), **kwargs
    )
    shards = [
        np.asarray(r["out"]).astype(np.float32).reshape(SEQ, B, NPC)
        for r in res.results
    ]
    full = np.concatenate(shards, axis=2)
    return full, res


def kernel(input_currents: np.ndarray) -> np.ndarray:
    full, _ = run_spmd(input_currents)
    return full


# revision 14
# speedup vs baseline: 1.1270x; 1.1270x over previous
"""Trainium2 Bass kernel for nn_ConstantCurrentLIFEncoder.

Reference semantics (norse constant_current_lif_encode, f32):
    v' = v + f32(0.1) * ((0 - v) + I);  z = (v' > 1);  v = v' - z*v'
for 100 steps from v=0, returning spikes [100, 32, 8192].

A spike resets v to exactly 0.0 (the initial state) and I is constant,
so each neuron's spike train is exactly periodic with period
    p = 1 + floor(ln((I-1)/I) / ln(1 - f32(0.1)))    (no spikes if I <= 1;
periods are clamped to 129 since anything > 100 never fires in-window).
Instead of a 100-step sequential scan the kernel computes p per neuron
in closed form (a handful of small ops on [128, 256]) and generates the
output from the periodicity:  z[t, n] = 1  iff  p_n | (t+1), evaluated as

    z[t, n] = floor((t+1.5) r_n) - floor((t+0.5) r_n),   r = 1/p

(the count of integers in ((t+0.5)r, (t+1.5)r], which is 0 or 1 since
r <= 1/7).  With x[t] = (t+0.5) r_n for t = 0..100 this is
z[t] = F[t+1] - F[t], F = floor(x).  Compute engines cannot read at a
+1 partition offset (the ISA cannot encode it), but DMA moves across
partitions freely, so the shifted copy G[t] = F[t+1] is produced by a
small SBUF->SBUF DMA on the ScalarE hardware queue (the output stream
runs on the Sync queue in parallel):

  * PE      : x = (t+0.5)-row (bf16) x r-row (bf16), t = 0..100; K=1
              outer products -> PSUM f32 [101, 512] per bank.  bf16
              runs 1 cycle/row (fp32 is 4) and the products are exact
              in f32; r's bf16 rounding shifts x by <= 0.196 r, margin
              to any decision boundary is 0.5 r.
  * floor   : F = RN_bf16(x + 143.5) = 144 + floor(x) (add-magic trick;
              exact: F in [144, 159], bf16 ulp there is 1).  Tiles are
              split between ScalarE (activation Copy + bias) and DVE
              (tensor_scalar add) to balance engine load.
  * shift   : G = F[1:101] via SBUF->SBUF DMA (partition shift).
  * diff    : z = G - F (tensor_tensor subtract, all-bf16 SBUF operands
              -> 2x DVE mode), some tiles on GpSimd.
  * DMA     : z -> DRAM as bf16 (half the bytes of f32); the host casts
              back to f32 (0.0/1.0 are exact in bf16).

PE K=1 operands must sit at partition bases {0,64}, so r is relayouted
from [partition, group] to two flat 16384-wide rows via a small DRAM
relayout (DMA transpose + SBUF->SBUF row gather), one-time cost.

Sharding: pure data parallel over the neuron axis; core c handles
columns [c*1024, (c+1)*1024) of the [32, 8192] input. No collectives.
"""

import math

import numpy as np

SEQ = 100
SEQ1 = SEQ + 1
B = 32
N = 8192
NCORES = 8
NPC = N // NCORES  # neurons (last-axis columns) per core
M = B * NPC  # flat neurons per core = 32768
P = 128
J = M // P  # 128-neuron groups per core = 256

# Effective per-step decay of (v - I): v' - I = (1 - f32(0.1)) * (v - I).
DECAY = 1.0 - float(np.float32(0.1))
INV_LN_D = float(np.float32(1.0 / math.log(DECAY)))
C23 = float(2.0**23)
# RN_bf16(x + 143.5) = 144 + floor(x) for x in (0, 16) away from integers:
# v = x + 143.5 and the result 144 + floor(x) <= 159 both stay inside the
# [128, 256) binade where bf16's ulp is exactly 1 (7 stored mantissa bits).
MAGIC = 143.5
KCLAMP = 128.0

TW = 2048  # neuron columns per compute tile (4 PSUM banks)
NT = M // TW  # 16 tiles per core
# tiles whose floor runs on DVE instead of ScalarE (ScalarE also issues
# the shift DMAs on its hardware queue, so it only takes a few floors)
ACT_FLOOR = {3, 7, 11, 15}
# tiles whose diff runs on GpSimd instead of DVE
GP_DIFF = {1, 3, 6, 8, 10, 13}

_CACHE: dict = {}


def _build_nc():
    import concourse.bacc as bacc
    import concourse.mybir as mybir
    from concourse import tile

    f32 = mybir.dt.float32
    bf16 = mybir.dt.bfloat16
    Alu = mybir.AluOpType
    Act = mybir.ActivationFunctionType

    nc = bacc.Bacc("TRN2", target_bir_lowering=False, debug=False)
    cur = nc.dram_tensor("cur", [P, J], f32, kind="ExternalInput")
    # (t+0.5) rows for t=0..100, replicated at partitions 0/64 (PE
    # quadrant bases; lhsT must share its base with rhs)
    trow = nc.dram_tensor("trow", [P, SEQ1], f32, kind="ExternalInput")
    out = nc.dram_tensor("out", [SEQ, M], bf16, kind="ExternalOutput")

    with tile.TileContext(nc) as tc:
        with (
            tc.tile_pool(name="consts", bufs=1) as consts,
            tc.tile_pool(name="pp", bufs=2, space="PSUM") as pp,
            tc.tile_pool(name="fp", bufs=3) as fp,
            tc.tile_pool(name="gp", bufs=2) as gp,
            tc.tile_pool(name="zp", bufs=2) as zp,
        ):
            cur_s = consts.tile([P, J], f32)
            trow_s = consts.tile([P, SEQ1], f32)
            trow16 = consts.tile([P, SEQ1], bf16)
            t1 = consts.tile([P, J], f32)
            t2 = consts.tile([P, J], f32)
            rbuf = consts.tile([P, J], f32)
            rbuf16 = consts.tile([P, J], bf16)
            rT = [
                consts.tile([P, P], bf16, name=f"rT{c}", tag=f"rT{c}")
                for c in range(J // P)
            ]
            rquad = consts.tile([65, M // 2], bf16)

            # Warm the ScalarE activation table (Ln) before the input DMA
            # lands so the ~2.5us table load overlaps the preamble DMAs.
            nc.gpsimd.memset(t1[:, 0:8], 1.0)
            nc.scalar.activation(t2[:, 0:8], t1[:, 0:8], Act.Ln)

            nc.sync.dma_start(cur_s[:], cur[:, :])
            nc.sync.dma_start(trow_s[:], trow[:, :])
            nc.scalar.activation(trow16[:], trow_s[:], Act.Copy)

            # ---- per-neuron period p, rate r = 1/p ----
            # Processed in two 128-column halves so the first half's r row
            # (neurons 0..16383) is ready ~5us earlier and the matmul stream
            # starts while the second half's chain still runs.  Each half's
            # columns of rbuf are exactly one DMA-transpose chunk.
            for h in range(2):
                cs = slice(h * P, (h + 1) * P)
                t1h, t2h = t1[:, cs], t2[:, cs]
                nc.vector.reciprocal(t1h, cur_s[:, cs])  # 1/I
                nc.vector.tensor_scalar(t2h, cur_s[:, cs], -1.0, None, Alu.add)
                nc.vector.tensor_tensor(t2h, t2h, t1h, Alu.mult)  # (I-1)/I
                nc.vector.tensor_scalar(t2h, t2h, 1e-38, None, Alu.max)
                nc.scalar.activation(t1h, t2h, Act.Ln)
                # k = ln(a)/ln(d), clamped so p <= 129 (never fires in-window)
                nc.vector.tensor_scalar(
                    t1h, t1h, INV_LN_D, KCLAMP, Alu.mult, Alu.min
                )
                # floor(k) via the round-to-nearest add-2^23 trick on k - 0.5
                nc.vector.tensor_scalar(
                    t1h, t1h, C23 - 0.5, C23, Alu.add, Alu.subtract
                )
                nc.vector.tensor_scalar(t2h, t1h, 1.0, None, Alu.add)  # p
                nc.vector.reciprocal(rbuf[:, cs], t2h)  # r = 1/p, f32
                # PE needs flat bf16 rows at quadrant bases: cast to bf16,
                # DMA-transpose, bounce through DRAM, reload the
                # 16384-neuron half into partition 64*h.
                nc.scalar.activation(rbuf16[:, cs], rbuf[:, cs], Act.Copy)
                nc.sync.dma_start_transpose(rT[h][:], rbuf16[:, cs])
                nc.sync.dma_start(rquad[64 * h : 64 * h + 1, :], rT[h][:, :])

            # ---- spike-train generation, output-layout [time, neuron] ----
            # Floors are issued as two half-tiles so the first half starts
            # while the PE is still filling the second (frees the PSUM slot
            # earlier, keeping the PE fed).
            HW2 = TW // 2
            for w in range(NT):
                half, off = divmod(w * TW, M // 2)
                b = 64 * half
                x = pp.tile([SEQ1, TW], f32, name="x", tag="x")
                for s in range(TW // 512):
                    nc.tensor.matmul(
                        x[:, s * 512 : (s + 1) * 512],
                        trow16[b : b + 1, 0:SEQ1],
                        rquad[b : b + 1, off + s * 512 : off + (s + 1) * 512],
                        start=True,
                        stop=True,
                    )
                fb = fp.tile([SEQ1, TW], bf16)
                for h in range(2):
                    fs = fb[:, h * HW2 : (h + 1) * HW2]
                    xsl = x[:, h * HW2 : (h + 1) * HW2]
                    if w in ACT_FLOOR:
                        nc.scalar.activation(fs, xsl, Act.Copy, bias=MAGIC)
                    else:
                        nc.vector.tensor_scalar(fs, xsl, MAGIC, None, Alu.add)
                g = gp.tile([SEQ, TW], bf16)
                nc.scalar.dma_start(g[:], fb[1:SEQ1, :])
                z = zp.tile([SEQ, TW], bf16)
                eng = nc.gpsimd if w in GP_DIFF else nc.vector
                eng.tensor_tensor(z[:], g[:], fb[0:SEQ, :], Alu.subtract)
                nc.sync.dma_start(out[0:SEQ, w * TW : (w + 1) * TW], z[:])

    nc.compile()
    return nc


def _get_nc():
    if "nc" not in _CACHE:
        _CACHE["nc"] = _build_nc()
    return _CACHE["nc"]


def _in_maps(input_currents: np.ndarray):
    trow = np.zeros((P, SEQ1), dtype=np.float32)
    for b in (0, 64):
        trow[b, :] = np.arange(SEQ1, dtype=np.float32) + np.float32(0.5)
    maps = []
    for c in range(NCORES):
        shard = np.asarray(input_currents[:, c * NPC : (c + 1) * NPC], dtype=np.float32)
        # device layout: cur[p, j] = flat neuron j*128 + p
        cur = np.ascontiguousarray(shard.reshape(M).reshape(J, P).T)
        maps.append({"cur": cur, "trow": trow})
    return maps


def run_spmd(input_currents: np.ndarray, **kwargs):
    from concourse.bass_utils import run_bass_kernel_spmd

    nc = _get_nc()
    res = run_bass_kernel_spmd(
        nc, _in_maps(input_currents), core_ids=list(range(NCORES)), **kwargs
    )
    shards = [
        np.asarray(r["out"]).astype(np.float32).reshape(SEQ, B, NPC)
        for r in res.results
    ]
    full = np.concatenate(shards, axis=2)
    return full, res


def kernel(input_currents: np.ndarray) -> np.ndarray:
    full, _ = run_spmd(input_currents)
    return full


# revision 15
# speedup vs baseline: 1.3585x; 1.2054x over previous
"""Trainium2 Bass kernel for nn_ConstantCurrentLIFEncoder.

Reference semantics (norse constant_current_lif_encode, f32):
    v' = v + f32(0.1) * ((0 - v) + I);  z = (v' > 1);  v = v' - z*v'
for 100 steps from v=0, returning spikes [100, 32, 8192].

A spike resets v to exactly 0.0 (the initial state) and I is constant,
so each neuron's spike train is exactly periodic with period
    p = 1 + floor(ln((I-1)/I) / ln(1 - f32(0.1)))    (no spikes if I <= 1;
periods are clamped to 129 since anything > 100 never fires in-window).
Instead of a 100-step sequential scan the kernel computes p per neuron
in closed form (a handful of small ops on [128, 256]) and generates the
output from the periodicity:  z[t, n] = 1  iff  p_n | (t+1), evaluated as

    z[t, n] = floor((t+1.5) r_n) - floor((t+0.5) r_n),   r = 1/p

(the count of integers in ((t+0.5)r, (t+1.5)r], which is 0 or 1 since
r <= 1/7).  With x[t] = (t+0.5) r_n for t = 0..100 this is
z[t] = F[t+1] - F[t], F = floor(x).  Compute engines cannot read at a
+1 partition offset (the ISA cannot encode it), but DMA moves across
partitions freely, so the shifted copy G[t] = F[t+1] is produced by a
small SBUF->SBUF DMA on the ScalarE hardware queue (the output stream
runs on the Sync queue in parallel):

  * PE      : x = (t+0.5)-row (bf16) x r-row (bf16), t = 0..100; K=1
              outer products -> PSUM f32 [101, 512] per bank.  bf16
              runs 1 cycle/row (fp32 is 4) and the products are exact
              in f32; r's bf16 rounding shifts x by <= 0.196 r, margin
              to any decision boundary is 0.5 r.
  * floor   : F = RN_bf16(x + 143.5) = 144 + floor(x) (add-magic trick;
              exact: F in [144, 159], bf16 ulp there is 1).  Tiles are
              split between ScalarE (activation Copy + bias) and DVE
              (tensor_scalar add) to balance engine load.
  * shift   : G = F[1:101] via SBUF->SBUF DMA (partition shift).
  * diff    : z = G - F (tensor_tensor subtract, all-bf16 SBUF operands
              -> 2x DVE mode), some tiles on GpSimd.
  * DMA     : z -> DRAM as bf16 (half the bytes of f32); the host casts
              back to f32 (0.0/1.0 are exact in bf16).

PE K=1 operands must sit at partition bases {0,64}, so r is relayouted
from [partition, group] to two flat 16384-wide rows via a small DRAM
relayout (DMA transpose + SBUF->SBUF row gather), one-time cost.

Sharding: pure data parallel over the neuron axis; core c handles
columns [c*1024, (c+1)*1024) of the [32, 8192] input. No collectives.
"""

import math

import numpy as np

SEQ = 100
SEQ1 = SEQ + 1
B = 32
N = 8192
NCORES = 8
NPC = N // NCORES  # neurons (last-axis columns) per core
M = B * NPC  # flat neurons per core = 32768
P = 128
J = M // P  # 128-neuron groups per core = 256

# Effective per-step decay of (v - I): v' - I = (1 - f32(0.1)) * (v - I).
DECAY = 1.0 - float(np.float32(0.1))
INV_LN_D = float(np.float32(1.0 / math.log(DECAY)))
C23 = float(2.0**23)
# RN_bf16(x + 143.5) = 144 + floor(x) for x in (0, 16) away from integers:
# v = x + 143.5 and the result 144 + floor(x) <= 159 both stay inside the
# [128, 256) binade where bf16's ulp is exactly 1 (7 stored mantissa bits).
MAGIC = 143.5
KCLAMP = 128.0

TW = 2048  # neuron columns per compute tile (4 PSUM banks)
NT = M // TW  # 16 tiles per core
# tiles whose floor runs on DVE instead of ScalarE (ScalarE also issues
# the shift DMAs on its hardware queue, so it only takes a few floors)
ACT_FLOOR = {3, 7, 11, 15}
# tiles whose diff runs on GpSimd instead of DVE
GP_DIFF = {1, 3, 6, 8, 10, 13}

_CACHE: dict = {}


def _build_nc():
    import concourse.bacc as bacc
    import concourse.mybir as mybir
    from concourse import tile

    f32 = mybir.dt.float32
    bf16 = mybir.dt.bfloat16
    Alu = mybir.AluOpType
    Act = mybir.ActivationFunctionType

    nc = bacc.Bacc("TRN2", target_bir_lowering=False, debug=False)
    cur = nc.dram_tensor("cur", [P, J], f32, kind="ExternalInput")
    # (t+0.5) rows for t=0..100, replicated at partitions 0/64 (PE
    # quadrant bases; lhsT must share its base with rhs)
    trow = nc.dram_tensor("trow", [P, SEQ1], f32, kind="ExternalInput")
    out = nc.dram_tensor("out", [SEQ, M], bf16, kind="ExternalOutput")

    with tile.TileContext(nc) as tc:
        with (
            tc.tile_pool(name="consts", bufs=1) as consts,
            tc.tile_pool(name="pp", bufs=2, space="PSUM") as pp,
            tc.tile_pool(name="fp", bufs=4) as fp,
            tc.tile_pool(name="gp", bufs=3) as gp,
            tc.tile_pool(name="zp", bufs=3) as zp,
        ):
            cur_s = consts.tile([P, J], f32)
            trow_s = consts.tile([P, SEQ1], f32)
            trow16 = consts.tile([P, SEQ1], bf16)
            t1 = consts.tile([P, J], f32)
            t2 = consts.tile([P, J], f32)
            rbuf = consts.tile([P, J], f32)
            rbuf16 = consts.tile([P, J], bf16)
            rT = [
                consts.tile([P, P], bf16, name=f"rT{c}", tag=f"rT{c}")
                for c in range(J // P)
            ]
            rquad = consts.tile([65, M // 2], bf16)

            # Warm the ScalarE activation table (Ln) before the input DMA
            # lands so the ~2.5us table load overlaps the preamble DMAs.
            nc.gpsimd.memset(t1[:, 0:8], 1.0)
            nc.scalar.activation(t2[:, 0:8], t1[:, 0:8], Act.Ln)

            nc.sync.dma_start(cur_s[:], cur[:, :])
            nc.sync.dma_start(trow_s[:], trow[:, :])
            nc.scalar.activation(trow16[:], trow_s[:], Act.Copy)

            # ---- per-neuron period p, rate r = 1/p ----
            # Processed in two 128-column halves so the first half's r row
            # (neurons 0..16383) is ready ~5us earlier and the matmul stream
            # starts while the second half's chain still runs.  Each half's
            # columns of rbuf are exactly one DMA-transpose chunk.
            for h in range(2):
                cs = slice(h * P, (h + 1) * P)
                t1h, t2h = t1[:, cs], t2[:, cs]
                nc.vector.reciprocal(t1h, cur_s[:, cs])  # 1/I
                nc.vector.tensor_scalar(t2h, cur_s[:, cs], -1.0, None, Alu.add)
                nc.vector.tensor_tensor(t2h, t2h, t1h, Alu.mult)  # (I-1)/I
                nc.vector.tensor_scalar(t2h, t2h, 1e-38, None, Alu.max)
                nc.scalar.activation(t1h, t2h, Act.Ln)
                # k = ln(a)/ln(d), clamped so p <= 129 (never fires in-window)
                nc.vector.tensor_scalar(
                    t1h, t1h, INV_LN_D, KCLAMP, Alu.mult, Alu.min
                )
                # floor(k) via the round-to-nearest add-2^23 trick on k - 0.5
                nc.vector.tensor_scalar(
                    t1h, t1h, C23 - 0.5, C23, Alu.add, Alu.subtract
                )
                nc.vector.tensor_scalar(t2h, t1h, 1.0, None, Alu.add)  # p
                nc.vector.reciprocal(rbuf[:, cs], t2h)  # r = 1/p, f32
                # PE needs flat bf16 rows at quadrant bases: cast to bf16,
                # DMA-transpose, bounce through DRAM, reload the
                # 16384-neuron half into partition 64*h.
                nc.scalar.activation(rbuf16[:, cs], rbuf[:, cs], Act.Copy)
                nc.sync.dma_start_transpose(rT[h][:], rbuf16[:, cs])
                nc.sync.dma_start(rquad[64 * h : 64 * h + 1, :], rT[h][:, :])

            # ---- spike-train generation, output-layout [time, neuron] ----
            # Floors are issued as two half-tiles so the first half starts
            # while the PE is still filling the second (frees the PSUM slot
            # earlier, keeping the PE fed).
            # Software-pipelined: tile w's diff + output DMA are emitted
            # during iteration w+1, so the shift-DMA's completion latency
            # never stalls the in-order DVE queue (which would block the
            # next tile's floors, the PSUM slots, and finally the PE).
            HW2 = TW // 2
            fbs, gs = {}, {}

            def diff_and_out(v):
                z = zp.tile([SEQ, TW], bf16)
                eng = nc.gpsimd if v in GP_DIFF else nc.vector
                eng.tensor_tensor(
                    z[:], gs[v][:], fbs[v][0:SEQ, :], Alu.subtract
                )
                nc.sync.dma_start(out[0:SEQ, v * TW : (v + 1) * TW], z[:])
                del fbs[v], gs[v]

            for w in range(NT):
                half, off = divmod(w * TW, M // 2)
                b = 64 * half
                x = pp.tile([SEQ1, TW], f32, name="x", tag="x")
                for s in range(TW // 512):
                    nc.tensor.matmul(
                        x[:, s * 512 : (s + 1) * 512],
                        trow16[b : b + 1, 0:SEQ1],
                        rquad[b : b + 1, off + s * 512 : off + (s + 1) * 512],
                        start=True,
                        stop=True,
                    )
                fb = fp.tile([SEQ1, TW], bf16)
                for h in range(2):
                    fs = fb[:, h * HW2 : (h + 1) * HW2]
                    xsl = x[:, h * HW2 : (h + 1) * HW2]
                    if w in ACT_FLOOR:
                        nc.scalar.activation(fs, xsl, Act.Copy, bias=MAGIC)
                    else:
                        nc.vector.tensor_scalar(fs, xsl, MAGIC, None, Alu.add)
                g = gp.tile([SEQ, TW], bf16)
                nc.scalar.dma_start(g[:], fb[1:SEQ1, :])
                fbs[w], gs[w] = fb, g
                if w > 0:
                    diff_and_out(w - 1)
            diff_and_out(NT - 1)

    nc.compile()
    return nc


def _get_nc():
    if "nc" not in _CACHE:
        _CACHE["nc"] = _build_nc()
    return _CACHE["nc"]


def _in_maps(input_currents: np.ndarray):
    trow = np.zeros((P, SEQ1), dtype=np.float32)
    for b in (0, 64):
        trow[b, :] = np.arange(SEQ1, dtype=np.float32) + np.float32(0.5)
    maps = []
    for c in range(NCORES):
        shard = np.asarray(input_currents[:, c * NPC : (c + 1) * NPC], dtype=np.float32)
        # device layout: cur[p, j] = flat neuron j*128 + p
        cur = np.ascontiguousarray(shard.reshape(M).reshape(J, P).T)
        maps.append({"cur": cur, "trow": trow})
    return maps


def run_spmd(input_currents: np.ndarray, **kwargs):
    from concourse.bass_utils import run_bass_kernel_spmd

    nc = _get_nc()
    res = run_bass_kernel_spmd(
        nc, _in_maps(input_currents), core_ids=list(range(NCORES)), **kwargs
    )
    shards = [
        np.asarray(r["out"]).astype(np.float32).reshape(SEQ, B, NPC)
        for r in res.results
    ]
    full = np.concatenate(shards, axis=2)
    return full, res


def kernel(input_currents: np.ndarray) -> np.ndarray:
    full, _ = run_spmd(input_currents)
    return full


# revision 16
# speedup vs baseline: 1.4424x; 1.0618x over previous
"""Trainium2 Bass kernel for nn_ConstantCurrentLIFEncoder.

Reference semantics (norse constant_current_lif_encode, f32):
    v' = v + f32(0.1) * ((0 - v) + I);  z = (v' > 1);  v = v' - z*v'
for 100 steps from v=0, returning spikes [100, 32, 8192].

A spike resets v to exactly 0.0 (the initial state) and I is constant,
so each neuron's spike train is exactly periodic with period
    p = 1 + floor(ln((I-1)/I) / ln(1 - f32(0.1)))    (no spikes if I <= 1;
periods are clamped to 129 since anything > 100 never fires in-window).
Instead of a 100-step sequential scan the kernel computes p per neuron
in closed form (a handful of small ops on [128, 256]) and generates the
output from the periodicity:  z[t, n] = 1  iff  p_n | (t+1), evaluated as

    z[t, n] = floor((t+1.5) r_n) - floor((t+0.5) r_n),   r = 1/p

(the count of integers in ((t+0.5)r, (t+1.5)r], which is 0 or 1 since
r <= 1/7).  With x[t] = (t+0.5) r_n for t = 0..100 this is
z[t] = F[t+1] - F[t], F = floor(x).  Compute engines cannot read at a
+1 partition offset (the ISA cannot encode it), but DMA moves across
partitions freely, so the shifted copy G[t] = F[t+1] is produced by a
small SBUF->SBUF DMA on the ScalarE hardware queue (the output stream
runs on the Sync queue in parallel):

  * PE      : x = (t+0.5)-row (bf16) x r-row (bf16), t = 0..100; K=1
              outer products -> PSUM f32 [101, 512] per bank.  bf16
              runs 1 cycle/row (fp32 is 4) and the products are exact
              in f32; r's bf16 rounding shifts x by <= 0.196 r, margin
              to any decision boundary is 0.5 r.
  * floor   : F = RN_bf16(x + 143.5) = 144 + floor(x) (add-magic trick;
              exact: F in [144, 159], bf16 ulp there is 1).  Tiles are
              split between ScalarE (activation Copy + bias) and DVE
              (tensor_scalar add) to balance engine load.
  * shift   : G = F[1:101] via SBUF->SBUF DMA (partition shift).
  * diff    : z = G - F (tensor_tensor subtract, all-bf16 SBUF operands
              -> 2x DVE mode), some tiles on GpSimd.
  * DMA     : z -> DRAM as bf16 (half the bytes of f32); the host casts
              back to f32 (0.0/1.0 are exact in bf16).

PE K=1 operands must sit at partition bases {0,64}, so r is relayouted
from [partition, group] to two flat 16384-wide rows via a small DRAM
relayout (DMA transpose + SBUF->SBUF row gather), one-time cost.

Sharding: pure data parallel over the neuron axis; core c handles
columns [c*1024, (c+1)*1024) of the [32, 8192] input. No collectives.
"""

import math

import numpy as np

SEQ = 100
SEQ1 = SEQ + 1
B = 32
N = 8192
NCORES = 8
NPC = N // NCORES  # neurons (last-axis columns) per core
M = B * NPC  # flat neurons per core = 32768
P = 128
J = M // P  # 128-neuron groups per core = 256

# Effective per-step decay of (v - I): v' - I = (1 - f32(0.1)) * (v - I).
DECAY = 1.0 - float(np.float32(0.1))
INV_LN_D = float(np.float32(1.0 / math.log(DECAY)))
C23 = float(2.0**23)
# RN_bf16(x + 143.5) = 144 + floor(x) for x in (0, 16) away from integers:
# v = x + 143.5 and the result 144 + floor(x) <= 159 both stay inside the
# [128, 256) binade where bf16's ulp is exactly 1 (7 stored mantissa bits).
MAGIC = 143.5
KCLAMP = 128.0

TW = 2048  # neuron columns per compute tile (4 PSUM banks)
NT = M // TW  # 16 tiles per core
# tiles whose floor runs on DVE instead of ScalarE (ScalarE also issues
# the shift DMAs on its hardware queue, so it only takes a few floors)
ACT_FLOOR = {0, 3, 6, 9, 12, 15}
# tiles whose diff runs on GpSimd instead of DVE
GP_DIFF = {1, 4, 7, 10, 13}

_CACHE: dict = {}


def _build_nc():
    import concourse.bacc as bacc
    import concourse.mybir as mybir
    from concourse import tile

    f32 = mybir.dt.float32
    bf16 = mybir.dt.bfloat16
    Alu = mybir.AluOpType
    Act = mybir.ActivationFunctionType

    nc = bacc.Bacc("TRN2", target_bir_lowering=False, debug=False)
    cur = nc.dram_tensor("cur", [P, J], f32, kind="ExternalInput")
    # (t+0.5) rows for t=0..100, replicated at partitions 0/64 (PE
    # quadrant bases; lhsT must share its base with rhs)
    trow = nc.dram_tensor("trow", [P, SEQ1], f32, kind="ExternalInput")
    out = nc.dram_tensor("out", [SEQ, M], bf16, kind="ExternalOutput")

    with tile.TileContext(nc) as tc:
        with (
            tc.tile_pool(name="consts", bufs=1) as consts,
            tc.tile_pool(name="pp", bufs=2, space="PSUM") as pp,
            tc.tile_pool(name="fp", bufs=6) as fp,
            tc.tile_pool(name="gp", bufs=5) as gp,
            tc.tile_pool(name="zp", bufs=4) as zp,
        ):
            cur_s = consts.tile([P, J], f32)
            trow_s = consts.tile([P, SEQ1], f32)
            trow16 = consts.tile([P, SEQ1], bf16)
            t1 = consts.tile([P, J], f32)
            t2 = consts.tile([P, J], f32)
            rbuf = consts.tile([P, J], f32)
            rbuf16 = consts.tile([P, J], bf16)
            rT = [
                consts.tile([P, P], bf16, name=f"rT{c}", tag=f"rT{c}")
                for c in range(J // P)
            ]
            rquad = consts.tile([65, M // 2], bf16)

            # Warm the ScalarE activation table (Ln) before the input DMA
            # lands so the ~2.5us table load overlaps the preamble DMAs.
            nc.gpsimd.memset(t1[:, 0:8], 1.0)
            nc.scalar.activation(t2[:, 0:8], t1[:, 0:8], Act.Ln)

            nc.sync.dma_start(cur_s[:], cur[:, :])
            nc.sync.dma_start(trow_s[:], trow[:, :])
            nc.scalar.activation(trow16[:], trow_s[:], Act.Copy)

            # ---- per-neuron period p, rate r = 1/p ----
            # Processed in two 128-column halves so the first half's r row
            # (neurons 0..16383) is ready ~5us earlier and the matmul stream
            # starts while the second half's chain still runs.  Each half's
            # columns of rbuf are exactly one DMA-transpose chunk.
            for h in range(2):
                cs = slice(h * P, (h + 1) * P)
                t1h, t2h = t1[:, cs], t2[:, cs]
                # ln((I-1)/I) = ln(I-1) - ln(I); the max() guards I <= 1
                # (ln(eps) makes k huge, clamped to 129 => never fires).
                nc.vector.tensor_scalar(
                    t2h, cur_s[:, cs], -1.0, 1e-38, Alu.add, Alu.max
                )
                nc.scalar.activation(t2h, t2h, Act.Ln)  # ln(I-1)
                nc.scalar.activation(t1h, cur_s[:, cs], Act.Ln)  # ln(I)
                nc.vector.tensor_tensor(t1h, t2h, t1h, Alu.subtract)
                # k = ln(a)/ln(d), clamped so p <= 129 (never fires in-window)
                nc.vector.tensor_scalar(
                    t1h, t1h, INV_LN_D, KCLAMP, Alu.mult, Alu.min
                )
                # floor(k) via the round-to-nearest add-2^23 trick on k - 0.5
                nc.vector.tensor_scalar(
                    t1h, t1h, C23 - 0.5, C23, Alu.add, Alu.subtract
                )
                nc.vector.tensor_scalar(t2h, t1h, 1.0, None, Alu.add)  # p
                nc.vector.reciprocal(rbuf[:, cs], t2h)  # r = 1/p, f32
                # PE needs flat bf16 rows at quadrant bases: cast to bf16,
                # DMA-transpose, bounce through DRAM, reload the
                # 16384-neuron half into partition 64*h.
                nc.scalar.activation(rbuf16[:, cs], rbuf[:, cs], Act.Copy)
                nc.sync.dma_start_transpose(rT[h][:], rbuf16[:, cs])
                nc.sync.dma_start(rquad[64 * h : 64 * h + 1, :], rT[h][:, :])

            # ---- spike-train generation, output-layout [time, neuron] ----
            # Floors are issued as two half-tiles so the first half starts
            # while the PE is still filling the second (frees the PSUM slot
            # earlier, keeping the PE fed).
            # Software-pipelined: tile w's diff + output DMA are emitted
            # during iteration w+1, so the shift-DMA's completion latency
            # never stalls the in-order DVE queue (which would block the
            # next tile's floors, the PSUM slots, and finally the PE).
            fbs, gs = {}, {}

            def diff_and_out(v):
                z = zp.tile([SEQ, TW], bf16)
                eng = nc.gpsimd if v in GP_DIFF else nc.vector
                eng.tensor_tensor(
                    z[:], gs[v][:], fbs[v][0:SEQ, :], Alu.subtract
                )
                nc.sync.dma_start(out[0:SEQ, v * TW : (v + 1) * TW], z[:])
                del fbs[v], gs[v]

            for w in range(NT):
                half, off = divmod(w * TW, M // 2)
                b = 64 * half
                x = pp.tile([SEQ1, TW], f32, name="x", tag="x")
                for s in range(TW // 512):
                    nc.tensor.matmul(
                        x[:, s * 512 : (s + 1) * 512],
                        trow16[b : b + 1, 0:SEQ1],
                        rquad[b : b + 1, off + s * 512 : off + (s + 1) * 512],
                        start=True,
                        stop=True,
                    )
                fb = fp.tile([SEQ1, TW], bf16)
                if w in ACT_FLOOR:
                    nc.scalar.activation(fb[:], x[:], Act.Copy, bias=MAGIC)
                else:
                    nc.vector.tensor_scalar(fb[:], x[:], MAGIC, None, Alu.add)
                g = gp.tile([SEQ, TW], bf16)
                nc.scalar.dma_start(g[:], fb[1:SEQ1, :])
                fbs[w], gs[w] = fb, g
                if w > 0:
                    diff_and_out(w - 1)
            diff_and_out(NT - 1)

    nc.compile()
    return nc


def _get_nc():
    if "nc" not in _CACHE:
        _CACHE["nc"] = _build_nc()
    return _CACHE["nc"]


def _in_maps(input_currents: np.ndarray):
    trow = np.zeros((P, SEQ1), dtype=np.float32)
    for b in (0, 64):
        trow[b, :] = np.arange(SEQ1, dtype=np.float32) + np.float32(0.5)
    maps = []
    for c in range(NCORES):
        shard = np.asarray(input_currents[:, c * NPC : (c + 1) * NPC], dtype=np.float32)
        # device layout: cur[p, j] = flat neuron j*128 + p
        cur = np.ascontiguousarray(shard.reshape(M).reshape(J, P).T)
        maps.append({"cur": cur, "trow": trow})
    return maps


def run_spmd(input_currents: np.ndarray, **kwargs):
    from concourse.bass_utils import run_bass_kernel_spmd

    nc = _get_nc()
    res = run_bass_kernel_spmd(
        nc, _in_maps(input_currents), core_ids=list(range(NCORES)), **kwargs
    )
    shards = [
        np.asarray(r["out"]).astype(np.float32).reshape(SEQ, B, NPC)
        for r in res.results
    ]
    full = np.concatenate(shards, axis=2)
    return full, res


def kernel(input_currents: np.ndarray) -> np.ndarray:
    full, _ = run_spmd(input_currents)
    return full
